# revision 25
# baseline (speedup 1.0000x reference)
"""VQ codebook argmax kernel for Trainium2 (8 NeuronCores, SPMD data-parallel).

Problem: x [2,96,48,48,48] fp32, prototypes [512,96] fp32.
Output: argmax_k cosine_sim(x[:, :, v], prototypes[k]) -> [2,48,48,48] int32.

Math notes:
  - argmax over k of (x_hat . p_hat_k) == argmax over k of (x . p_hat_k):
    per-voxel positive scaling (1/||x||) never changes the argmax, so x is
    NOT normalized (saves a full partition-dim reduction on device).
  - prototypes ARE normalized (host side, fp32, same formula as reference).
  - matmul precision: one fp16 main matmul per tile (fp16 runs at bf16
    rate on the PE, verified on HW), plus on EVERY OTHER tile a second
    fp16 matmul folding both first-order corrections: sims' = xh@Ph + u@W
    with xh=f16(x), xl=x-xh (exact in fp32), u=f16(xl + s*xh), Ph=f16(pn),
    Pl=pn-Ph, W=f16((1+s)(Ph+Pl/s)), s=2^-6. Algebra for corrected tiles:
    sims' = (1+s+s^2)*sims + O(2^-20.7); the global scale never changes
    the argmax. Uncorrected tiles carry plain-f16 error (~2^-15.6).
    Cost: 1.5 PE passes/tile. Measured on the actual input: 58 argmax
    flips vs the fp32 reference (rel err 1.1e-2, gate is 2e-2).
  - argmax on device: single fused custom DVE op per 128-voxel tile.
    The 512 sims live in one PSUM bank [128, 512]; columns are permuted so
    column q holds proto 511-2q and column 256+q holds proto 510-2q.
    The op consumes two 256-wide streams (Src0 = cols 0:256 from PSUM,
    Src1 = cols 256:512 via an SBUF copy done by the Scalar engine) and
    folds: m = max(a,b); rec = (m == running_max(m)); wo = (m == b);
    pos = (2(j+1) - 1024) + wo; accum MAX of select(rec, pos, -FLT_MAX).
    The accumulated A encodes the winner: k* = -(A + 511), and the
    (j asc, wo) priority order makes ties resolve EXACTLY like np.argmax
    (first occurrence) - fuzz-verified 20000 cases.
"""

import numpy as np
import ml_dtypes
from contextlib import ExitStack

import concourse.bass as bass
import concourse.bacc as bacc
import concourse.tile as tile
from concourse import mybir
from concourse.bass_utils import run_bass_kernel_spmd

# ----------------------------------------------------------------------------
# problem constants (hardcoded per contract)
N_CORES = 8
B, C, D, H, W = 2, 96, 48, 48, 48
N_VOX = B * D * H * W            # 221184
VOX_PER_CORE = N_VOX // N_CORES  # 27648
K = 512                          # prototypes
TILE_V = 128                     # voxels per matmul tile (PSUM partition dim)
TILES_PER_CORE = VOX_PER_CORE // TILE_V  # 216
CHUNK_V = 1024                   # voxels per DMA chunk
CHUNKS = VOX_PER_CORE // CHUNK_V  # 27
TILES_PER_CHUNK = CHUNK_V // TILE_V  # 8

_BF16 = ml_dtypes.bfloat16
S_COMB = 2.0 ** -6               # scale folding the two correction terms

# ----------------------------------------------------------------------------
# custom DVE op registration (argmax fold over paired streams)

_VQARG_NAME = "VQ_ARGMAX_ANT"
_VQARG_OP = None


def _vqarg_reference(in0, in1, c0, c1, c2):
    a = np.asarray(in0, np.float32)
    b = np.asarray(in1, np.float32)
    p = a.shape[0]
    a2 = a.reshape(p, -1)
    b2 = b.reshape(p, -1)
    c1v = float(c1) if np.isscalar(c1) or isinstance(c1, float) else np.asarray(c1, np.float32)
    m = np.maximum(a2, b2)
    r = np.maximum.accumulate(m, axis=1)
    rec = m == r
    wo = (m == b2).astype(np.float32)
    n = a2.shape[1]
    s2 = (np.float32(-float(c2)) + np.float32(c1v) * np.arange(1, n + 1, dtype=np.float32))
    pos = s2[None, :] + wo
    body = np.where(rec, pos, np.float32(-3.4028235e38)).astype(np.float32)
    acc = body.max(axis=1, keepdims=True)
    return body.reshape(a.shape), acc


def _register_vqarg():
    global _VQARG_OP
    if _VQARG_OP is not None:
        return _VQARG_OP
    from concourse.dve_spec import (
        Spec, Src0, Src1, C1, C2, Zero, MaxNeg, eq, select, scan, AluOp, maxx,
        lower, _has_src1 as has_src1,
    )
    from concourse import dve_ops
    from concourse.dve_uop import DveOpSpec

    m = maxx(Src0, Src1)
    r = scan(AluOp.MAX, m)
    rec = eq(m, r)
    wo = eq(m, Src1)
    s2 = scan(AluOp.ADD, C1, init=Zero - C2)
    pos = s2 + wo
    spec = Spec(
        body=select(rec, pos, MaxNeg),
        accum=AluOp.MAX,
        reference=_vqarg_reference,
    )

    if _VQARG_NAME in dve_ops._SUB_OPCODE_FOR_NAME:
        row = dve_ops._SUB_OPCODE_FOR_NAME[_VQARG_NAME]
    else:
        row = max(dve_ops._SUB_OPCODE_FOR_NAME.values()) + 1
        assert row < 0x20, "no free custom-DVE opcode row"
        dve_ops._SUB_OPCODE_FOR_NAME[_VQARG_NAME] = row

    shas = {}
    for ver in ("v3", "v4"):
        s = DveOpSpec(
            name=_VQARG_NAME,
            opcode=row,
            uops=lower(spec, ver=ver),
            rd1_en=has_src1(spec),
        )
        shas[ver] = s.sha(ver)

    op = dve_ops.DveOp(_VQARG_NAME, spec, subdim=False, uops_sha=shas)
    if all(o.name != _VQARG_NAME for o in dve_ops.OPS):
        dve_ops.OPS.append(op)
    dve_ops.CUSTOM_DVE_SPECS[_VQARG_NAME] = spec
    _VQARG_OP = op
    return op


# ----------------------------------------------------------------------------
# device program

_PROG = None

import os as _os
ACT_COPY = _os.environ.get("VQ_ACT_COPY", "1") == "1"
N_WARMUP = int(_os.environ.get("VQ_WARMUP", "8"))
GROUP = 2                        # tiles per PSUM group / scalar staging copy


def build_program(vox_per_core=VOX_PER_CORE, chunk_v=CHUNK_V):
    """Build + compile the per-core SPMD Bass program. Returns (nc, meta)."""
    vqarg = _register_vqarg()
    dt = mybir.dt
    chunks = vox_per_core // chunk_v
    tiles_per_chunk = chunk_v // TILE_V
    n_tiles = vox_per_core // TILE_V

    nc = bacc.Bacc(
        "TRN2", target_bir_lowering=False, debug=False, num_devices=N_CORES
    )
    n_corr = (n_tiles + 1) // 2
    xh_d = nc.dram_tensor("xh", [C, vox_per_core], dt.float16, kind="ExternalInput").ap()
    xl_d = nc.dram_tensor("xl", [C, n_corr * TILE_V], dt.float16, kind="ExternalInput").ap()
    ph_d = nc.dram_tensor("pht", [C, K], dt.float16, kind="ExternalInput").ap()
    pl_d = nc.dram_tensor("plt", [C, K], dt.float16, kind="ExternalInput").ap()
    out_d = nc.dram_tensor("outA", [TILE_V, n_tiles], dt.float32, kind="ExternalOutput").ap()

    with tile.TileContext(nc) as tc, ExitStack() as ctx:
        cpool = ctx.enter_context(tc.tile_pool(name="const", bufs=1))
        xpool = ctx.enter_context(tc.tile_pool(name="x", bufs=4))
        ppool = ctx.enter_context(tc.tile_pool(name="psum", bufs=4, space="PSUM"))
        spool = ctx.enter_context(tc.tile_pool(name="scr", bufs=4))
        hpool = ctx.enter_context(tc.tile_pool(name="half", bufs=4))
        apool = ctx.enter_context(tc.tile_pool(name="acc", bufs=1))

        # PE warmup on memset data: no input dependency, so it starts as
        # soon as the engines come up (~6us) and releases the HAM clock
        # throttle (~3.4us sustained PE activity -> 2.4 GHz) right as the
        # first real matmul's inputs land. 8 x 512-col f16 matmuls at the
        # throttled 1.2 GHz = 3.4us exactly. Results are discarded.
        if N_WARMUP:
            wsrc = cpool.tile([TILE_V, K], dt.float16)
            nc.gpsimd.memset(wsrc[:], 0.0)
            wps = ppool.tile([TILE_V, GROUP, K], dt.float32, tag="ps")
            for _ in range(N_WARMUP):
                nc.tensor.matmul(wps[:, 0:1, :], wsrc[:, 0:TILE_V], wsrc[:],
                                 start=True, stop=True)

        # tables go on the gpsimd DMA queue so they land in parallel with the
        # first x chunk on the sync queue (PE needs both before matmul 0)
        ph_sb = cpool.tile([C, K], dt.float16)
        nc.gpsimd.dma_start(ph_sb[:], ph_d[:])
        pl_sb = cpool.tile([C, K], dt.float16)
        nc.gpsimd.dma_start(pl_sb[:], pl_d[:])

        jsb = apool.tile([TILE_V, n_tiles], dt.float32)

        # ramp-in: small leading chunks so the first matmul starts sooner
        if chunks > 2:
            sizes = [256, 256, 512] + [chunk_v] * (chunks - 1)
        else:
            sizes = [chunk_v] * chunks
        assert sum(sizes) == vox_per_core
        base = 0
        tid = 0
        pend = []
        for cv in sizes:
            xh_sb = xpool.tile([C, cv], dt.float16, tag="xh")
            nc.sync.dma_start(xh_sb[:], xh_d[:, base:base + cv])
            # correction stream u: packed for even tiles only (tid % 2 == 0)
            t0 = base // TILE_V
            ctiles = [t for t in range(t0, t0 + cv // TILE_V) if t % 2 == 0]
            xl_sb = xpool.tile([C, len(ctiles) * TILE_V], dt.float16, tag="xl")
            u0 = (ctiles[0] // 2) * TILE_V
            nc.sync.dma_start(xl_sb[:], xl_d[:, u0:u0 + len(ctiles) * TILE_V])
            base += cv
            for t in range(cv // TILE_V):
                lhsu = None
                if (t0 + t) % 2 == 0:
                    ci = ctiles.index(t0 + t)
                    lhsu = xl_sb[:, ci * TILE_V:(ci + 1) * TILE_V]
                pend.append((tid, xh_sb[:, t * TILE_V:(t + 1) * TILE_V], lhsu))
                tid += 1
                if len(pend) < GROUP:
                    continue
                psg = ppool.tile([TILE_V, GROUP, K], dt.float32, tag="ps")
                # issue the uncorrected (single-matmul) tile first: its PSUM
                # WAR semaphore arrives just-in-time (ew~30ns), so giving it
                # the group-leading slot hides the wait under the previous
                # group's streaming instead of stretching this group.
                pend.sort(key=lambda p: p[2] is not None)
                for j, (gt, lhs_h, lhs_l) in enumerate(pend):
                    if lhs_l is None:
                        nc.tensor.matmul(psg[:, j:j + 1, :], lhs_h, ph_sb[:],
                                         start=True, stop=True)
                    else:
                        nc.tensor.matmul(psg[:, j:j + 1, :], lhs_h, ph_sb[:],
                                         start=True, stop=False)
                        nc.tensor.matmul(psg[:, j:j + 1, :], lhs_l, pl_sb[:],
                                         start=False, stop=True)
                # one Scalar ACTIVATE stages the group's second halves
                # ([128, GROUP, 256] strided across banks), paying the
                # ~352-elem Scalar pipeline-fill cost once per group.
                # (flat tile: 2D in1 slices keep the TTSS struct / imm2 slot)
                half = hpool.tile([TILE_V, GROUP * (K // 2)], dt.float32)
                if tid == n_tiles:
                    # final group: per-tile copies shorten the tail's serial
                    # chain (first fold starts after a 256-col copy instead
                    # of waiting for the full group copy)
                    for j in range(GROUP):
                        nc.scalar.copy(half[:, j * (K // 2):(j + 1) * (K // 2)],
                                       psg[:, j:j + 1, K // 2:K])
                else:
                    nc.scalar.copy(half[:], psg[:, :, K // 2:K])
                for j, (gt, _, _) in enumerate(pend):
                    scr = spool.tile([TILE_V, K // 2], dt.float32)
                    nc.vector._custom_dve(
                        vqarg,
                        out=scr[:],
                        in0=psg[:, j:j + 1, 0:K // 2],
                        in1=half[:, j * (K // 2):(j + 1) * (K // 2)],
                        s0=0.0,
                        s1=2.0,
                        imm2=1024.0,
                        accum_out=jsb[:, gt:gt + 1],
                    )
                pend = []
                if n_tiles > 32 and tid == n_tiles - 4:
                    # drain most results early (hidden under remaining tiles)
                    # so only a 2KB DMA sits after the last fold
                    nc.sync.dma_start(out_d[:, :tid], jsb[:, :tid])
        assert tid == n_tiles and not pend
        split = n_tiles - 4 if n_tiles > 32 else 0
        nc.sync.dma_start(out_d[:, split:], jsb[:, split:])

    nc.compile()
    return nc


def _get_program():
    global _PROG
    if _PROG is None:
        _PROG = build_program()
    return _PROG


# ----------------------------------------------------------------------------
# host-side prep + entry point

def _bf16_split(a):
    """fp16 hi part + fp16 combined-correction part u = f16(xl + s*xh)."""
    hi = a.astype(np.float16)
    lo = ((a - hi.astype(np.float32)) + S_COMB * hi.astype(np.float32)).astype(
        np.float16
    )
    return hi, lo


def _pack_u(u_core):
    """Keep the correction stream for even tiles only (tid % 2 == 0)."""
    return np.ascontiguousarray(
        u_core.reshape(C, -1, TILE_V)[:, ::2, :].reshape(C, -1)
    )


def _prep_prototypes(prototypes):
    pn = prototypes / np.maximum(
        np.linalg.norm(prototypes, axis=1, keepdims=True), 1e-12
    )
    pn = pn.astype(np.float32)
    q = np.arange(K // 2)
    perm = np.concatenate([511 - 2 * q, 510 - 2 * q])  # col layout for VQARG
    pc = pn[perm]
    ph = pc.astype(np.float16)
    pl = pc - ph.astype(np.float32)
    w = ((1.0 + S_COMB) * (ph.astype(np.float32) + pl / S_COMB)).astype(np.float16)
    pht = np.ascontiguousarray(ph.T)  # [96, 512] f16
    plt = np.ascontiguousarray(w.T)   # [96, 512] f16 (the combined W table)
    return pht, plt


def kernel(x, prototypes):
    x = np.asarray(x, np.float32)
    prototypes = np.asarray(prototypes, np.float32)

    # [2,96,48,48,48] -> [96, 221184] with global voxel = b*110592 + dhw
    xt = np.ascontiguousarray(
        x.reshape(B, C, D * H * W).transpose(1, 0, 2).reshape(C, N_VOX)
    )
    xh, xl = _bf16_split(xt)
    pht, plt = _prep_prototypes(prototypes)

    in_maps = []
    for c in range(N_CORES):
        sl = slice(c * VOX_PER_CORE, (c + 1) * VOX_PER_CORE)
        in_maps.append({
            "xh": np.ascontiguousarray(xh[:, sl]),
            "xl": _pack_u(xl[:, sl]),
            "pht": pht,
            "plt": plt,
        })

    nc = _get_program()
    res = None
    last_err = None
    for attempt in range(3):
        try:
            res = run_bass_kernel_spmd(nc, in_maps, list(range(N_CORES)))
            break
        except Exception as e:  # transient axon/NRT hiccups self-recover
            last_err = e
            import time as _time
            _time.sleep(20 * (attempt + 1))
    if res is None:
        raise last_err

    outs = []
    for c in range(N_CORES):
        A = np.asarray(res.results[c]["outA"], np.float32)  # [128, 216]
        kidx = -(A + np.float32(511.0))                     # exact small ints
        outs.append(kidx.T.reshape(-1))                     # voxel = t*128 + p
    full = np.concatenate(outs)
    return full.reshape(B, D, H, W).astype(np.int32)

